# revision 1
# baseline (speedup 1.0000x reference)
"""Trainium2 Bass kernel for ConditionalLoRALinear.

Reference computation (f32):
    base = x @ W.T + b                      # [B,S,Do]
    lora = (x @ A.T) @ B.T * 2.0            # rank-8
    out  = base + lora * (ids == 7)         # per-token gate

Sharding over 8 NeuronCores: 2 token-halves x 4 d_out-quarters.
Each core holds its W-quarter (transposed, 16 MB) resident in SBUF and
streams its x-half (transposed) through in 128-token strips.  All
matmuls run as float32r (FP22-truncated f32) at full PE rate.  The
per-token LoRA gate (x2 scaling folded in) is precomputed on host as a
{0,2} float vector and applied on-device to the rank-8 activations
before the rank-8 matmul accumulates into the same PSUM banks as the
base matmul.  Bias is added during the PSUM->SBUF copy on DVE.
"""

import sys

for _p in ("/opt/trn_rl_repo",):
    if _p not in sys.path:
        sys.path.insert(0, _p)

from contextlib import ExitStack

import numpy as np

import concourse.bass as bass
import concourse.mybir as mybir
import concourse.tile as tile
from concourse import bacc
from concourse.bass import ts
from concourse.bass_utils import run_bass_kernel_spmd
from concourse.masks import make_identity

F32 = mybir.dt.float32
F32R = mybir.dt.float32r
BF16 = mybir.dt.bfloat16

B, S, DI, DO = 4, 4096, 4096, 4096
TOK = B * S              # 16384 tokens
NCORES = 8
TH = TOK // 2            # tokens per core (half)        = 8192
DQ = DO // 4             # d_out per core (quarter)      = 1024
P = 128                  # partition / strip size
KC = DI // P             # k-chunks                      = 32
NSTRIP = TH // P         # token strips per core         = 64
OC = DQ // 512           # 512-wide output chunks        = 2
COMP_TOKEN_ID = 7
SCALING = 2.0


import os

# bisect/debug flag: full | base | xa | transp (default full)
KMODE = os.environ.get("KMODE", "full")


def _build_nc():
    nc = bacc.Bacc(
        "TRN2",
        target_bir_lowering=False,
        debug=False,
        enable_asserts=True,
        num_devices=NCORES,
    )

    xT_d = nc.dram_tensor("xT", [NSTRIP, P, KC * P], F32R, kind="ExternalInput").ap()
    wT_d = nc.dram_tensor("wT", [DI, DQ], F32R, kind="ExternalInput").ap()
    aT_d = nc.dram_tensor("aT", [P, KC * 8], F32R, kind="ExternalInput").ap()
    bT_d = nc.dram_tensor("bT", [8, DQ], BF16, kind="ExternalInput").ap()
    bias_d = nc.dram_tensor("biasr", [P, DQ], F32, kind="ExternalInput").ap()
    mask_d = nc.dram_tensor("maskp", [P, NSTRIP], F32, kind="ExternalInput").ap()
    out_d = nc.dram_tensor("out", [TH, DQ], F32, kind="ExternalOutput").ap()


    with tile.TileContext(nc) as tc, ExitStack() as ctx:
        consts = ctx.enter_context(tc.tile_pool(name="consts", bufs=1))
        xpool = ctx.enter_context(tc.tile_pool(name="xp", bufs=2))
        opool = ctx.enter_context(tc.tile_pool(name="op", bufs=2))
        spool = ctx.enter_context(tc.tile_pool(name="sp", bufs=1))
        psum = ctx.enter_context(tc.tile_pool(name="ps", bufs=2, space="PSUM"))

        # ---- resident constants ----
        # W chunks alternate between the two descriptor-generation paths
        # (sync=HWDGE, gpsimd=SWDGE) so the initial 16 MB load isn't
        # serialized on a single queue ahead of the first compute.
        at = consts.tile([P, KC * 8], F32R, name="at", tag="at")
        nc.sync.dma_start(at[:], aT_d[:, :])
        bt = consts.tile([8, DQ], BF16, name="bt", tag="bt")
        nc.sync.dma_start(bt[:], bT_d[:, :])
        biast = consts.tile([P, DQ], F32, name="biast", tag="biast")
        nc.sync.dma_start(biast[:], bias_d[:, :])
        maskt = consts.tile([P, NSTRIP], F32, name="maskt", tag="maskt")
        nc.sync.dma_start(maskt[:], mask_d[:, :])
        WB = 4  # K-chunks per W tile / DMA
        w_tiles = []
        for wb in range(KC // WB):
            wt = consts.tile([P, WB, DQ], F32R, name=f"w{wb}", tag=f"w{wb}")
            eng = nc.sync if wb % 2 == 0 else nc.gpsimd
            eng.dma_start(
                wt[:], wT_d[ts(wb, WB * P), :].rearrange("(b p) o -> p b o", p=P)
            )
            w_tiles.append(wt)

        def epi_a(st):
            """copy the strip's transposed rank-8 activations out of PSUM."""
            s, out_ps, xaT_ps = st
            if KMODE == "base":
                return None
            xaT = spool.tile([8, P], BF16, name="xaT", tag="xaT", bufs=2)
            nc.vector.tensor_copy(xaT[:], xaT_ps[:])
            return xaT

        def epi_b(st, xaT):
            """rank-8 update (own PSUM), gate+bias on DVE, store."""
            s, out_ps, xaT_ps = st
            ob = opool.tile([P, DQ], F32, name="ob", tag="ob")
            for j in range(OC):
                if KMODE == "full":
                    lora_ps = psum.tile(
                        [P, 512], F32, name=f"lora_ps{j}", tag=f"pl{j}", bufs=1
                    )
                    nc.tensor.matmul(
                        lora_ps[:], xaT[:], bt[:, ts(j, 512)], start=True, stop=True
                    )
                    nc.vector.tensor_scalar_mul(
                        ob[:, ts(j, 512)], lora_ps[:], maskt[:, s : s + 1]
                    )
                    nc.vector.tensor_add(
                        ob[:, ts(j, 512)], ob[:, ts(j, 512)], out_ps[j][:]
                    )
                else:
                    nc.vector.tensor_copy(ob[:, ts(j, 512)], out_ps[j][:])
                nc.vector.tensor_add(
                    ob[:, ts(j, 512)], ob[:, ts(j, 512)], biast[:, ts(j, 512)]
                )
            nc.sync.dma_start(out_d[ts(s, P), :], ob[:])

        prev = None
        prev_xaT = None
        for s in range(NSTRIP):
            xt = xpool.tile([P, KC, P], F32R, name="xt", tag="xt")
            nc.scalar.dma_start(xt[:], xT_d[s].rearrange("p (c t) -> p c t", t=P))
            out_ps = [
                psum.tile([P, 512], F32, name=f"out_ps{j}", tag=f"po{j}")
                for j in range(OC)
            ]
            # rank-8 activations computed directly transposed: [r, tokens]
            xaT_ps = psum.tile([8, P], F32, name="xaT_ps", tag="pxa")
            epia_c = min(8, KC - 1)
            epib_c = min(16, KC - 1)
            for c in range(KC):
                if c == epia_c and prev is not None:
                    prev_xaT = epi_a(prev)
                # previous strip's epilogue mid-stream: its PSUM slots are
                # released well before the next strip needs them, so the PE
                # never idles across a strip boundary (keeps HAM at 8/8).
                if c == epib_c and prev is not None:
                    epi_b(prev, prev_xaT)
                    prev = None
                lhsT = xt[:, c, :]
                for j in range(OC):
                    nc.tensor.matmul(
                        out_ps[j][:],
                        lhsT,
                        w_tiles[c // WB][:, c % WB, ts(j, 512)],
                        start=(c == 0),
                        stop=(c == KC - 1),
                    )
                if KMODE != "base":
                    nc.tensor.matmul(
                        xaT_ps[:],
                        at[:, ts(c, 8)],
                        lhsT,
                        start=(c == 0),
                        stop=(c == KC - 1),
                    )
            prev = (s, out_ps, xaT_ps)

        prev_xaT = epi_a(prev)
        epi_b(prev, prev_xaT)

    nc.compile()
    return nc


_NC_CACHE = None


def _get_nc():
    global _NC_CACHE
    if _NC_CACHE is None:
        _NC_CACHE = _build_nc()
    return _NC_CACHE


def _make_in_maps(x, ids, W, b, lora_A, lora_B):
    x2 = np.ascontiguousarray(np.asarray(x, dtype=np.float32).reshape(TOK, DI))
    xT = np.ascontiguousarray(x2.T)                      # [DI, TOK]
    WT = np.ascontiguousarray(np.asarray(W, dtype=np.float32).T)   # [DI, DO]
    BT = np.ascontiguousarray(np.asarray(lora_B, dtype=np.float32).T)  # [8, DO]
    AT = np.asarray(lora_A, dtype=np.float32).T          # [DI, 8]
    # [DI, 8] -> [P, KC*8] with aT[p, c*8+r] = A[r, c*128+p]
    aT_pre = np.ascontiguousarray(
        AT.reshape(KC, P, 8).transpose(1, 0, 2).reshape(P, KC * 8)
    )
    bias = np.asarray(b, dtype=np.float32)
    maskf = (np.asarray(ids).reshape(TOK) == COMP_TOKEN_ID).astype(
        np.float32
    ) * SCALING

    # strip-contiguous layout: xprep[s, p, c*128+t] = x[h*TH + s*128+t, c*128+p]
    xT_half = [
        np.ascontiguousarray(
            xT[:, h * TH : (h + 1) * TH]
            .reshape(KC, P, NSTRIP, P)
            .transpose(2, 1, 0, 3)
            .reshape(NSTRIP, P, KC * P)
        )
        for h in range(2)
    ]
    mask_half = [
        np.ascontiguousarray(maskf[h * TH : (h + 1) * TH].reshape(NSTRIP, P).T)
        for h in range(2)
    ]
    wT_q = [np.ascontiguousarray(WT[:, q * DQ : (q + 1) * DQ]) for q in range(4)]
    import ml_dtypes

    bT_q = [
        np.ascontiguousarray(BT[:, q * DQ : (q + 1) * DQ]).astype(ml_dtypes.bfloat16)
        for q in range(4)
    ]
    bias_q = [
        np.ascontiguousarray(
            np.broadcast_to(bias[q * DQ : (q + 1) * DQ], (P, DQ))
        )
        for q in range(4)
    ]

    in_maps = []
    for c in range(NCORES):
        h, q = c // 4, c % 4
        in_maps.append(
            {
                "xT": xT_half[h],
                "wT": wT_q[q],
                "aT": aT_pre,
                "bT": bT_q[q],
                "biasr": bias_q[q],
                "maskp": mask_half[h],
            }
        )
    return in_maps


LDWOPT = os.environ.get("LDWOPT", "0") == "1"


def kernel(x, ids, W, b, lora_A, lora_B):
    nc = _get_nc()
    in_maps = _make_in_maps(x, ids, W, b, lora_A, lora_B)
    import concourse.bass_utils as _bu

    _orig_rc = _bu.run_command

    def _rc(argv, **kw):
        argv = [
            "--enable-ldw-opt=true" if a == "--enable-ldw-opt=false" else a
            for a in argv
        ]
        return _orig_rc(argv, **kw)

    if LDWOPT:
        _bu.run_command = _rc
    try:
        results = run_bass_kernel_spmd(nc, in_maps, core_ids=list(range(NCORES)))
    finally:
        _bu.run_command = _orig_rc
    out = np.empty((TOK, DO), dtype=np.float32)
    for c in range(NCORES):
        h, q = c // 4, c % 4
        out[h * TH : (h + 1) * TH, q * DQ : (q + 1) * DQ] = results.results[c]["out"]
    return out.reshape(B, S, DO)


if __name__ == "__main__":
    rng = np.random.default_rng(0)
    x = rng.standard_normal((B, S, DI), dtype=np.float32)
    ids = rng.integers(0, 64, size=(B, S)).astype(np.int64)
    W = rng.standard_normal((DO, DI), dtype=np.float32) / np.sqrt(DI)
    b = (rng.standard_normal(DO) * 0.02).astype(np.float32)
    lora_A = rng.standard_normal((8, DI), dtype=np.float32) / np.sqrt(DI)
    lora_B = (rng.standard_normal((DO, 8)) * 0.02).astype(np.float32)
    out = kernel(x, ids, W, b, lora_A, lora_B)
    print(out.shape, out.dtype, float(np.abs(out).mean()))



# revision 3
# speedup vs baseline: 1.3969x; 1.3969x over previous
"""Trainium2 Bass kernel for ConditionalLoRALinear.

Reference computation (f32):
    base = x @ W.T + b                      # [B,S,Do]
    lora = (x @ A.T) @ B.T * 2.0            # rank-8
    out  = base + lora * (ids == 7)         # per-token gate

Sharding over 8 NeuronCores: 2 token-halves x 4 d_out-quarters.
Each core holds its W-quarter (transposed, bf16, 8 MB) resident in
SBUF and streams its x-half (transposed, bf16) through in 128-token
strips.  All matmuls run in bf16 (full PE rate, same as float32r, but
half the HBM traffic and SBUF footprint) with f32 PSUM accumulation;
bf16 input rounding keeps the result well inside the 2e-2 relative
error budget.

Only ~1/64 of tokens are gated on (ids == COMP_TOKEN_ID), so the host
swaps the masked tokens into the first `nlora` (~2) strips of each
half (an involutive column swap touching only ~2*128 tokens); the
rank-8 LoRA path then runs on just those strips instead of all 64.
(128-wide matmuls pay a 4x PE penalty in f32r but run at full rate in
bf16.)  The per-token {0,2} gate is applied on DVE and the rank-8
update is added during the epilogue.  The output swap is undone on
host.
"""

import sys

for _p in ("/opt/trn_rl_repo",):
    if _p not in sys.path:
        sys.path.insert(0, _p)

from contextlib import ExitStack

import numpy as np

import concourse.bass as bass
import concourse.mybir as mybir
import concourse.tile as tile
from concourse import bacc
from concourse.bass import ts
from concourse.bass_utils import run_bass_kernel_spmd

F32 = mybir.dt.float32
BF16 = mybir.dt.bfloat16

B, S, DI, DO = 4, 4096, 4096, 4096
TOK = B * S              # 16384 tokens
NCORES = 8
TH = TOK // 2            # tokens per core (half)        = 8192
DQ = DO // 4             # d_out per core (quarter)      = 1024
P = 128                  # partition / strip size
KC = DI // P             # k-chunks                      = 32
NSTRIP = TH // P         # token strips per core         = 64
OC = DQ // 512           # 512-wide output chunks        = 2
COMP_TOKEN_ID = 7
SCALING = 2.0


def _build_nc(nlora):
    nc = bacc.Bacc(
        "TRN2",
        target_bir_lowering=False,
        debug=False,
        enable_asserts=True,
        num_devices=NCORES,
    )

    xT_d = nc.dram_tensor("xT", [NSTRIP, P, KC * P], BF16, kind="ExternalInput").ap()
    wT_d = nc.dram_tensor("wT", [DI, DQ], BF16, kind="ExternalInput").ap()
    a8_d = nc.dram_tensor("a8", [P, KC * 8], BF16, kind="ExternalInput").ap()
    bT_d = nc.dram_tensor("bT", [8, DQ], BF16, kind="ExternalInput").ap()
    bias_d = nc.dram_tensor("biasr", [P, DQ], F32, kind="ExternalInput").ap()
    mask_d = nc.dram_tensor("maskp", [P, NSTRIP], F32, kind="ExternalInput").ap()
    out_d = nc.dram_tensor("out", [TH, DQ], F32, kind="ExternalOutput").ap()

    with tile.TileContext(nc) as tc, ExitStack() as ctx:
        consts = ctx.enter_context(tc.tile_pool(name="consts", bufs=1))
        xpool = ctx.enter_context(tc.tile_pool(name="xp", bufs=3))
        opool = ctx.enter_context(tc.tile_pool(name="op", bufs=2))
        spool = ctx.enter_context(tc.tile_pool(name="sp", bufs=1))
        psum = ctx.enter_context(tc.tile_pool(name="ps", bufs=2, space="PSUM"))

        # ---- resident constants ----
        at8 = consts.tile([P, KC * 8], BF16, name="at8", tag="at8")
        nc.sync.dma_start(at8[:], a8_d[:, :])
        bt = consts.tile([8, DQ], BF16, name="bt", tag="bt")
        nc.sync.dma_start(bt[:], bT_d[:, :])
        biast = consts.tile([P, DQ], F32, name="biast", tag="biast")
        nc.sync.dma_start(biast[:], bias_d[:, :])
        maskt = consts.tile([P, NSTRIP], F32, name="maskt", tag="maskt")
        nc.sync.dma_start(maskt[:], mask_d[:, :])

        # W chunks alternate between the two descriptor-generation paths
        # (sync=HWDGE, gpsimd=SWDGE) so the initial 8 MB load isn't
        # serialized on a single queue ahead of the first compute.
        WB = 4  # K-chunks per W tile / DMA
        w_tiles = []
        for wb in range(KC // WB):
            wt = consts.tile([P, WB, DQ], BF16, name=f"w{wb}", tag=f"w{wb}")
            eng = nc.sync if wb % 2 == 0 else nc.gpsimd
            eng.dma_start(
                wt[:], wT_d[ts(wb, WB * P), :].rearrange("(b p) o -> p b o", p=P)
            )
            w_tiles.append(wt)

        def epi_a(st):
            """copy the strip's transposed rank-8 activations out of PSUM."""
            s, out_ps, xaT_ps = st
            if xaT_ps is None:
                return None
            xaT = spool.tile([8, P], BF16, name="xaT", tag="xaT", bufs=2)
            nc.vector.tensor_copy(xaT[:], xaT_ps[:])
            return xaT

        def epi_b(st, xaT):
            """rank-8 update (own PSUM), gate+bias on DVE, store."""
            s, out_ps, _ = st
            ob = opool.tile([P, DQ], F32, name="ob", tag="ob")
            for j in range(OC):
                if xaT is not None:
                    lora_ps = psum.tile(
                        [P, 512], F32, name=f"lora_ps{j}", tag=f"pl{j}", bufs=1
                    )
                    nc.tensor.matmul(
                        lora_ps[:], xaT[:], bt[:, ts(j, 512)], start=True, stop=True
                    )
                    nc.vector.tensor_scalar_mul(
                        ob[:, ts(j, 512)], lora_ps[:], maskt[:, s : s + 1]
                    )
                    nc.vector.tensor_add(
                        ob[:, ts(j, 512)], ob[:, ts(j, 512)], out_ps[j][:]
                    )
                    nc.vector.tensor_add(
                        ob[:, ts(j, 512)], ob[:, ts(j, 512)], biast[:, ts(j, 512)]
                    )
                else:
                    nc.vector.tensor_add(
                        ob[:, ts(j, 512)], out_ps[j][:], biast[:, ts(j, 512)]
                    )
            nc.sync.dma_start(out_d[ts(s, P), :], ob[:])

        prev = None
        prev_xaT = None
        epia_c = min(8, KC - 1)
        epib_c = min(16, KC - 1)
        for s in range(NSTRIP):
            xt = xpool.tile([P, KC, P], BF16, name="xt", tag="xt")
            nc.scalar.dma_start(xt[:], xT_d[s].rearrange("p (c t) -> p c t", t=P))
            lora_strip = s < nlora
            out_ps = [
                psum.tile([P, 512], F32, name=f"out_ps{j}", tag=f"po{j}")
                for j in range(OC)
            ]
            xaT_ps = (
                psum.tile([8, P], F32, name="xaT_ps", tag="pxa")
                if lora_strip
                else None
            )
            for c in range(KC):
                if c == epia_c and prev is not None:
                    prev_xaT = epi_a(prev)
                # previous strip's epilogue mid-stream: its PSUM slots are
                # released well before the next strip needs them, so the PE
                # never idles across a strip boundary.
                if c == epib_c and prev is not None:
                    epi_b(prev, prev_xaT)
                    prev = None
                lhsT = xt[:, c, :]
                for j in range(OC):
                    nc.tensor.matmul(
                        out_ps[j][:],
                        lhsT,
                        w_tiles[c // WB][:, c % WB, ts(j, 512)],
                        start=(c == 0),
                        stop=(c == KC - 1),
                    )
                if lora_strip:
                    # rank-8 activations, transposed [r, tokens]; bf16
                    # moving operand keeps this at full PE rate.
                    nc.tensor.matmul(
                        xaT_ps[:],
                        at8[:, ts(c, 8)],
                        lhsT,
                        start=(c == 0),
                        stop=(c == KC - 1),
                    )
            prev = (s, out_ps, xaT_ps)

        prev_xaT = epi_a(prev)
        epi_b(prev, prev_xaT)

    nc.compile()
    return nc


_NC_CACHE = {}


def _get_nc(nlora):
    if nlora not in _NC_CACHE:
        _NC_CACHE[nlora] = _build_nc(nlora)
    return _NC_CACHE[nlora]


def _prep_host(x, ids):
    """Cast x to bf16, transpose, and swap masked tokens to the front of
    each half.

    Returns (xT [DI,TOK] bf16 with swapped columns, permuted {0,2} mask,
    per-half swap index pairs, nlora strip count)."""
    import ml_dtypes

    x2 = np.asarray(x, dtype=np.float32).reshape(TOK, DI).astype(ml_dtypes.bfloat16)
    xT = np.ascontiguousarray(x2.T)  # [DI, TOK]
    maskf = (np.asarray(ids).reshape(TOK) == COMP_TOKEN_ID).astype(
        np.float32
    ) * SCALING
    swaps = []
    counts = []
    for h in range(2):
        mh = maskf[h * TH : (h + 1) * TH]
        midx = np.nonzero(mh > 0)[0]
        k = len(midx)
        counts.append(k)
        need_move = midx[midx >= k]
        front_free = np.nonzero(mh[:k] == 0)[0]
        assert len(need_move) == len(front_free)
        swaps.append((front_free, need_move))
    nlora = max(1, max((k + P - 1) // P for k in counts))
    nlora = min(nlora, NSTRIP)

    maskp = maskf.copy()
    for h, (a, b) in enumerate(swaps):
        if len(a):
            ga = h * TH + a
            gb = h * TH + b
            tmp = xT[:, ga].copy()
            xT[:, ga] = xT[:, gb]
            xT[:, gb] = tmp
            mtmp = maskp[ga].copy()
            maskp[ga] = maskp[gb]
            maskp[gb] = mtmp
    return xT, maskp, swaps, nlora


def _make_in_maps(xT, maskp, W, b, lora_A, lora_B):
    import ml_dtypes

    WT = np.ascontiguousarray(
        np.asarray(W, dtype=np.float32).T.astype(ml_dtypes.bfloat16)
    )  # [DI, DO]
    BT = np.ascontiguousarray(np.asarray(lora_B, dtype=np.float32).T)  # [8, DO]
    AT = np.asarray(lora_A, dtype=np.float32).T  # [DI, 8]
    # [DI, 8] -> [P, KC*8] with a8[p, c*8+r] = A[r, c*128+p]
    a8_pre = np.ascontiguousarray(
        AT.reshape(KC, P, 8).transpose(1, 0, 2).reshape(P, KC * 8)
    ).astype(ml_dtypes.bfloat16)
    bias = np.asarray(b, dtype=np.float32)

    # strip-contiguous layout: xprep[s, p, c*128+t] = x[h*TH + s*128+t, c*128+p]
    xT_half = [
        np.ascontiguousarray(
            xT[:, h * TH : (h + 1) * TH]
            .reshape(KC, P, NSTRIP, P)
            .transpose(2, 1, 0, 3)
            .reshape(NSTRIP, P, KC * P)
        )
        for h in range(2)
    ]
    mask_half = [
        np.ascontiguousarray(maskp[h * TH : (h + 1) * TH].reshape(NSTRIP, P).T)
        for h in range(2)
    ]
    wT_q = [np.ascontiguousarray(WT[:, q * DQ : (q + 1) * DQ]) for q in range(4)]
    bT_q = [
        np.ascontiguousarray(BT[:, q * DQ : (q + 1) * DQ]).astype(ml_dtypes.bfloat16)
        for q in range(4)
    ]
    bias_q = [
        np.ascontiguousarray(np.broadcast_to(bias[q * DQ : (q + 1) * DQ], (P, DQ)))
        for q in range(4)
    ]

    in_maps = []
    for c in range(NCORES):
        h, q = c // 4, c % 4
        in_maps.append(
            {
                "xT": xT_half[h],
                "wT": wT_q[q],
                "a8": a8_pre,
                "bT": bT_q[q],
                "biasr": bias_q[q],
                "maskp": mask_half[h],
            }
        )
    return in_maps


def kernel(x, ids, W, b, lora_A, lora_B):
    xT, maskp, swaps, nlora = _prep_host(x, ids)
    nc = _get_nc(nlora)
    in_maps = _make_in_maps(xT, maskp, W, b, lora_A, lora_B)
    results = run_bass_kernel_spmd(nc, in_maps, core_ids=list(range(NCORES)))
    out = np.empty((TOK, DO), dtype=np.float32)
    for c in range(NCORES):
        h, q = c // 4, c % 4
        out[h * TH : (h + 1) * TH, q * DQ : (q + 1) * DQ] = results.results[c]["out"]
    # undo the involutive token swap
    for h, (a, b_) in enumerate(swaps):
        if len(a):
            ga = h * TH + a
            gb = h * TH + b_
            tmp = out[ga].copy()
            out[ga] = out[gb]
            out[gb] = tmp
    return out.reshape(B, S, DO)


if __name__ == "__main__":
    rng = np.random.default_rng(0)
    x = rng.standard_normal((B, S, DI), dtype=np.float32)
    ids = rng.integers(0, 64, size=(B, S)).astype(np.int64)
    W = rng.standard_normal((DO, DI), dtype=np.float32) / np.sqrt(DI)
    b = (rng.standard_normal(DO) * 0.02).astype(np.float32)
    lora_A = rng.standard_normal((8, DI), dtype=np.float32) / np.sqrt(DI)
    lora_B = (rng.standard_normal((DO, 8)) * 0.02).astype(np.float32)
    out = kernel(x, ids, W, b, lora_A, lora_B)
    print(out.shape, out.dtype, float(np.abs(out).mean()))
